# revision 1
# baseline (speedup 1.0000x reference)
"""CQAttention Trainium2 kernel.

Full inputs: C (64,256,1024), Q (64,256,256), c_mask (64,1024) [all-ones],
q_mask (64,256) [all-ones], w (768,).  Output: (64, 1024, 1024) fp32.

Sharding: data-parallel over batch, 8 batches per core on 8 cores.

Math per batch (Ct = C^T (c,d), Qt = Q^T (q,d)):
  S[c,q] = (Ct w1)[c] + (Qt w2)[q] + sum_d Ct[c,d] w3[d] Qt[q,d]
  E = exp(S)            (|S| <~ 8 so no max subtraction needed)
  r[c] = sum_q E,  s[q] = sum_c E         (masks are all-ones)
  S1 = E/r (rows), S2 = E/s (cols)
  A  = S1 @ Qt          -> computed as A^T  = Qt^T @ (E^T/r)
  T  = S2^T @ Ct = diag(1/s) (E^T @ Ct)
  Bm = S1 @ T           -> computed as Bm^T = T^T @ (E^T/r)
  out = [Ct; A; Ct*A; Ct*Bm]^T  (i.e. (4d, c) layout = [C; A^T; C*A^T; C*Bm^T])

All matmuls run in float32r (TF32-like) on the PE at full speed.
"""

import sys

for _p in ("/opt/trn_rl_repo",):
    if _p not in sys.path:
        sys.path.insert(0, _p)

import numpy as np
from contextlib import ExitStack

import concourse.bass as bass
import concourse.mybir as mybir
import concourse.tile as tile
from concourse.bass_utils import run_bass_kernel_spmd

F32 = mybir.dt.float32
F32R = mybir.dt.float32r
EXP = mybir.ActivationFunctionType.Exp

N_CORES = 8
B_FULL, D, LC, LQ = 64, 256, 1024, 256
BPC = B_FULL // N_CORES  # batches per core
KT = D // 128            # 2 contraction tiles over d
CT_N = LC // 128         # 8 c-tiles
QT_N = LQ // 128         # 2 q-tiles


def split_multi_waits(nc):
    """Walrus in this container accepts at most one sync-wait command per
    instruction; hoist extras onto single-wait drain nops just before."""
    n_new = 0
    for fn in nc.m.functions:
        for blk in fn.blocks:
            out_list = []
            changed = False
            for inst in blk.instructions:
                si = inst.sync_info
                if si is not None and si.on_wait and len(si.on_wait) > 1:
                    waits = list(si.on_wait)
                    for w in waits[:-1]:
                        nop = mybir.InstDrain(
                            name=f"I-waitsplit-{n_new}", ins=[], outs=[]
                        )
                        n_new += 1
                        nop.engine = inst.engine
                        nop.sync_info = mybir.SyncInfo(on_wait=[w], on_update=[])
                        out_list.append(nop)
                    inst.sync_info = mybir.SyncInfo(
                        on_wait=[waits[-1]], on_update=list(si.on_update)
                    )
                    changed = True
                out_list.append(inst)
            if changed:
                blk.instructions = out_list
    return n_new


def build_module(n_batches=BPC, rounds=1):
    nc = bass.Bass()
    C_d = nc.declare_dram_parameter("C", [n_batches, D, LC], F32, isOutput=False)
    Qf_d = nc.declare_dram_parameter("Qf", [n_batches, D, LQ], F32R, isOutput=False)
    Ct_d = nc.declare_dram_parameter("Ct", [n_batches, LC, D + 2], F32R, isOutput=False)
    Qt_d = nc.declare_dram_parameter("Qt", [n_batches, LQ, D], F32R, isOutput=False)
    bcols_d = nc.declare_dram_parameter("bcols", [n_batches, 128, CT_N + QT_N], F32, isOutput=False)
    brows_d = nc.declare_dram_parameter("brows", [n_batches, 1, LC + LQ], F32R, isOutput=False)
    wcols_d = nc.declare_dram_parameter("wcols", [128, 6], F32, isOutput=False)
    ones_d = nc.declare_dram_parameter("onesv", [128, 1], F32, isOutput=False)
    out_d = nc.declare_dram_parameter(
        "out", [n_batches, 4 * D, LC], F32, isOutput=True
    )

    with tile.TileContext(nc) as tc, ExitStack() as ctx:
        cpool = ctx.enter_context(tc.tile_pool(name="const", bufs=1))
        spool = ctx.enter_context(tc.tile_pool(name="sbuf", bufs=2))
        ppool = ctx.enter_context(tc.tile_pool(name="psum", bufs=2, space="PSUM"))

        # ---- per-core constants ----
        wcols = cpool.tile([128, 6], F32, name="wcols")
        nc.sync.dma_start(wcols[:], wcols_d[:])
        onesv = cpool.tile([128, 1], F32, name="onesv")
        nc.sync.dma_start(onesv[:], ones_d[:])
        onesrow = cpool.tile([1, 128], F32, name="onesrow")
        nc.vector.memset(onesrow[:], 1.0)
        onesA_r = cpool.tile([1, 128], F32R, name="onesA_r")
        nc.scalar.copy(onesA_r[:], onesrow[:])
        ones128f = cpool.tile([128, 128], F32, name="ones128f")
        nc.vector.memset(ones128f[:], 1.0)
        ones128_r = cpool.tile([128, 128], F32R, name="ones128_r")
        nc.scalar.copy(ones128_r[:], ones128f[:])

        for _round in range(rounds):
          for b in range(n_batches):
            # ---------------- loads ----------------
            C_sb = spool.tile([128, KT, LC], F32, name="C_sb", tag="C_sb", bufs=3)
            nc.sync.dma_start(C_sb[:], C_d[b].rearrange("(k p) c -> p k c", p=128))
            Qf = spool.tile([128, KT, LQ], F32R, name="Qf", tag="Qf", bufs=3)
            nc.sync.dma_start(Qf[:], Qf_d[b].rearrange("(k p) q -> p k q", p=128))
            Ct = spool.tile([128, CT_N, D + 2], F32R, name="Ct", tag="Ct")
            nc.sync.dma_start(Ct[:], Ct_d[b].rearrange("(t p) d -> p t d", p=128))
            Qt = spool.tile([128, QT_N, D], F32R, name="Qt", tag="Qt")
            nc.sync.dma_start(Qt[:], Qt_d[b].rearrange("(t p) d -> p t d", p=128))
            bcols = spool.tile([128, CT_N + QT_N], F32, name="bcols", tag="bcols")
            nc.sync.dma_start(bcols[:], bcols_d[b])
            brows = spool.tile([1, LC + LQ], F32R, name="brows", tag="brows")
            nc.sync.dma_start(brows[:], brows_d[b])
            b1col = bcols[:, 0:CT_N]
            b2col = bcols[:, CT_N : CT_N + QT_N]
            b1row_r = brows[0:1, 0:LC]
            b2row_r = brows[0:1, LC : LC + LQ]

            # ---------------- Cw3 = C * w3 (f32r) ----------------
            Cw3 = spool.tile([128, KT, LC], F32R, name="Cw3", tag="Cw3", bufs=3)
            for k in range(KT):
                nc.vector.tensor_scalar_mul(
                    Cw3[:, k, :], C_sb[:, k, :], wcols[:, 4 + k : 5 + k]
                )

            # ---------------- S (c,q) -> E, row sums r ----------------
            E = spool.tile([128, CT_N, LQ], F32R, name="E", tag="E", bufs=3)
            for i in range(CT_N):
                ps = ppool.tile([128, LQ], F32, name="ps", tag="s")
                for k in range(KT):
                    nc.tensor.matmul(
                        ps[:],
                        Cw3[:, k, i * 128 : (i + 1) * 128],
                        Qf[:, k, :],
                        start=(k == 0),
                        stop=False,
                    )
                nc.tensor.matmul(
                    ps[:], onesA_r[:], b2row_r, start=False, stop=True
                )
                nc.scalar.activation(
                    E[:, i, :], ps[:], EXP, bias=b1col[:, i : i + 1]
                )

            # ---------------- St (q,c) -> Et -> Ent = Et/r ----------------
            Et = spool.tile([128, QT_N, LC], F32R, name="Et", tag="Et")
            for qt in range(QT_N):
                for nh in range(2):
                    pst = ppool.tile([128, 512], F32, name="pst", tag="st")
                    for k in range(KT):
                        nc.tensor.matmul(
                            pst[:],
                            Qf[:, k, qt * 128 : (qt + 1) * 128],
                            Cw3[:, k, nh * 512 : (nh + 1) * 512],
                            start=(k == 0),
                            stop=False,
                        )
                    nc.tensor.matmul(
                        pst[:],
                        onesA_r[:],
                        brows[0:1, nh * 512 : (nh + 1) * 512],
                        start=False,
                        stop=True,
                    )
                    nc.scalar.activation(
                        Et[:, qt, nh * 512 : (nh + 1) * 512],
                        pst[:],
                        EXP,
                        bias=b2col[:, qt : qt + 1],
                    )

            # r broadcast: ones128^T @ Et sums over q for every output row
            Ent = spool.tile([128, QT_N, LC], F32R, name="Ent", tag="Ent")
            invr_bc = spool.tile([128, LC], F32, name="invr_bc", tag="invr_bc")
            for nh in range(2):
                pbc = ppool.tile([128, 512], F32, name="pbc", tag="ab", bufs=3)
                for qt in range(QT_N):
                    nc.tensor.matmul(
                        pbc[:],
                        ones128_r[:],
                        Et[:, qt, nh * 512 : (nh + 1) * 512],
                        start=(qt == 0),
                        stop=(qt == QT_N - 1),
                    )
                nc.vector.reciprocal(
                    invr_bc[:, nh * 512 : (nh + 1) * 512], pbc[:]
                )
                for qt in range(QT_N):
                    nc.vector.tensor_mul(
                        Ent[:, qt, nh * 512 : (nh + 1) * 512],
                        Et[:, qt, nh * 512 : (nh + 1) * 512],
                        invr_bc[:, nh * 512 : (nh + 1) * 512],
                    )

            # ---------------- U = E^T @ [Ct|1]  -> T = U/s ----------------
            T = spool.tile([128, QT_N, D], F32R, name="T", tag="T")
            invs = spool.tile([128, QT_N], F32, name="invs", tag="invs")
            for qt in range(QT_N):
                pu = ppool.tile([128, D + 2], F32, name="pu", tag="u", bufs=1)
                for i in range(CT_N):
                    nc.tensor.matmul(
                        pu[:],
                        E[:, i, qt * 128 : (qt + 1) * 128],
                        Ct[:, i, :],
                        start=(i == 0),
                        stop=(i == CT_N - 1),
                    )
                nc.vector.reciprocal(invs[:, qt : qt + 1], pu[:, D : D + 1])
                nc.vector.tensor_scalar_mul(
                    T[:, qt, :], pu[:, 0:D], invs[:, qt : qt + 1]
                )

            # ---------------- outputs ----------------
            nc.gpsimd.dma_start(
                out_d[b, 0:D, :].rearrange("(k p) c -> p k c", p=128), C_sb[:]
            )

            o2st = spool.tile([128, KT, LC], F32, name="o2st", tag="o2st")
            o3st = spool.tile([128, KT, LC], F32, name="o3st", tag="o3st")
            for dt in range(KT):
                for nh in range(2):
                    pa = ppool.tile([128, 512], F32, name="pa", tag="ab", bufs=3)
                    for qt in range(QT_N):
                        nc.tensor.matmul(
                            pa[:],
                            Qt[:, qt, dt * 128 : (dt + 1) * 128],
                            Ent[:, qt, nh * 512 : (nh + 1) * 512],
                            start=(qt == 0),
                            stop=(qt == QT_N - 1),
                        )
                    nc.scalar.copy(o2st[:, dt, nh * 512 : (nh + 1) * 512], pa[:])
                    nc.vector.tensor_mul(
                        o3st[:, dt, nh * 512 : (nh + 1) * 512],
                        C_sb[:, dt, nh * 512 : (nh + 1) * 512],
                        o2st[:, dt, nh * 512 : (nh + 1) * 512],
                    )
            nc.gpsimd.dma_start(
                out_d[b, D : 2 * D, :].rearrange("(k p) c -> p k c", p=128), o2st[:]
            )
            nc.gpsimd.dma_start(
                out_d[b, 2 * D : 3 * D, :].rearrange("(k p) c -> p k c", p=128), o3st[:]
            )

            o4st = spool.tile([128, KT, LC], F32, name="o4st", tag="o4st")
            for dt in range(KT):
                for nh in range(2):
                    pm = ppool.tile([128, 512], F32, name="pm", tag="ab", bufs=3)
                    for qt in range(QT_N):
                        nc.tensor.matmul(
                            pm[:],
                            T[:, qt, dt * 128 : (dt + 1) * 128],
                            Ent[:, qt, nh * 512 : (nh + 1) * 512],
                            start=(qt == 0),
                            stop=(qt == QT_N - 1),
                        )
                    nc.vector.tensor_mul(
                        o4st[:, dt, nh * 512 : (nh + 1) * 512],
                        C_sb[:, dt, nh * 512 : (nh + 1) * 512],
                        pm[:],
                    )
            nc.gpsimd.dma_start(
                out_d[b, 3 * D : 4 * D, :].rearrange("(k p) c -> p k c", p=128), o4st[:]
            )

    split_multi_waits(nc)
    return nc


def rne12(x):
    """Round fp32 to f32r (11 mantissa bits, round-to-nearest-even)."""
    u = np.ascontiguousarray(x, dtype=np.float32).view(np.uint32).astype(np.uint64)
    lsb = (u >> np.uint64(12)) & np.uint64(1)
    u = (u + np.uint64(0x7FF) + lsb) & np.uint64(0xFFFFF000)
    return u.astype(np.uint32).view(np.float32)


def host_prep(C, Q, w):
    """Host-side packing: transposes, bias vectors, f32r pre-rounding."""
    B = C.shape[0]
    w1, w2, w3 = w[:D], w[D:2 * D], w[2 * D:]
    Ct = np.empty((B, LC, D + 2), np.float32)
    Ct[:, :, :D] = rne12(C.transpose(0, 2, 1))
    Ct[:, :, D:] = 1.0
    Qt = rne12(Q.transpose(0, 2, 1))
    Qf = rne12(Q)
    b1 = np.einsum("bdc,d->bc", C, w1).astype(np.float32)       # (B, LC)
    b2 = np.einsum("bdq,d->bq", Q, w2).astype(np.float32)       # (B, LQ)
    bcols = np.concatenate([
        b1.reshape(B, CT_N, 128).transpose(0, 2, 1),
        b2.reshape(B, QT_N, 128).transpose(0, 2, 1),
    ], axis=2).astype(np.float32).copy()                         # (B, 128, 10)
    brows = rne12(np.concatenate([b1, b2], axis=1))[:, None, :]  # (B, 1, 1280)
    return dict(Ct=Ct, Qt=Qt, Qf=Qf, bcols=bcols, brows=brows)


def _make_consts():
    ident = np.eye(128, dtype=np.float32)
    onesv = np.ones((128, 1), dtype=np.float32)
    return ident, onesv


def _wcols(w):
    # (128, 6): [:,0:2]=w1 halves, [:,2:4]=w2 halves, [:,4:6]=w3 halves
    w = np.asarray(w, dtype=np.float32)
    w1, w2, w3 = w[:D], w[D : 2 * D], w[2 * D :]
    cols = np.zeros((128, 6), dtype=np.float32)
    for k in range(KT):
        cols[:, 0 + k] = w1[k * 128 : (k + 1) * 128]
        cols[:, 2 + k] = w2[k * 128 : (k + 1) * 128]
        cols[:, 4 + k] = w3[k * 128 : (k + 1) * 128]
    return cols


_NC_CACHE = {}


def _get_module(n_batches=BPC, rounds=1):
    key = (n_batches, rounds)
    if key not in _NC_CACHE:
        _NC_CACHE[key] = build_module(n_batches, rounds)
    return _NC_CACHE[key]


def run_on_cores(C, Q, w, n_batches=BPC, n_cores=N_CORES, **spmd_kwargs):
    nc = _get_module(n_batches)
    ident, onesv = _make_consts()
    wcols = _wcols(w)
    prep = host_prep(np.asarray(C, np.float32), np.asarray(Q, np.float32),
                     np.asarray(w, np.float32))
    in_maps = []
    for c in range(n_cores):
        sl = slice(c * n_batches, (c + 1) * n_batches)
        m = {"C": np.ascontiguousarray(C[sl]), "wcols": wcols, "onesv": onesv}
        for k in ("Ct", "Qt", "Qf", "bcols", "brows"):
            m[k] = np.ascontiguousarray(prep[k][sl])
        in_maps.append(m)
    res = run_bass_kernel_spmd(nc, in_maps, list(range(n_cores)), **spmd_kwargs)
    return res


def timed_run(C, Q, w, iters=4, n_batches=BPC, n_cores=N_CORES, rounds=1):
    """Time the NEFF execution on 8 cores via PJRT with device-resident
    inputs; returns (best_seconds, per_iter_list)."""
    import time
    import jax
    from jax.experimental.shard_map import shard_map
    from jax.sharding import Mesh, PartitionSpec, NamedSharding
    from concourse import bass2jax
    from concourse.bass2jax import _bass_exec_p, partition_id_tensor, install_neuronx_cc_hook

    nc = _get_module(n_batches, rounds)
    install_neuronx_cc_hook()

    ident, onesv = _make_consts()
    wcols = _wcols(w)
    prep = host_prep(np.asarray(C, np.float32), np.asarray(Q, np.float32),
                     np.asarray(w, np.float32))
    in_maps = []
    for c in range(n_cores):
        sl = slice(c * n_batches, (c + 1) * n_batches)
        m = {"C": np.ascontiguousarray(C[sl]), "wcols": wcols, "onesv": onesv}
        for k in ("Ct", "Qt", "Qf", "bcols", "brows"):
            m[k] = np.ascontiguousarray(prep[k][sl])
        in_maps.append(m)

    partition_name = nc.partition_id_tensor.name if nc.partition_id_tensor else None
    in_names, out_names, out_avals, zero_outs = [], [], [], []
    for alloc in nc.m.functions[0].allocations:
        if not isinstance(alloc, mybir.MemoryLocationSet):
            continue
        name = alloc.memorylocations[0].name
        if alloc.kind == "ExternalInput":
            if name != partition_name:
                in_names.append(name)
        elif alloc.kind == "ExternalOutput":
            shape = tuple(alloc.tensor_shape)
            dtype = mybir.dt.np(alloc.dtype)
            out_names.append(name)
            out_avals.append(jax.core.ShapedArray(shape, dtype))
            zero_outs.append(np.zeros(shape, dtype))
    n_params = len(in_names)
    n_outs = len(out_avals)
    all_names = list(in_names) + list(out_names)
    if partition_name is not None:
        all_names.append(partition_name)

    def _body(*args):
        operands = list(args)
        if partition_name is not None:
            operands.append(partition_id_tensor())
        outs = _bass_exec_p.bind(
            *operands,
            out_avals=tuple(out_avals),
            in_names=tuple(all_names),
            out_names=tuple(out_names),
            lowering_input_output_aliases=(),
            sim_require_finite=True,
            sim_require_nnan=True,
            nc=nc,
        )
        return tuple(outs)

    devices = jax.devices()[:n_cores]
    mesh = Mesh(np.asarray(devices), ("core",))
    spec = PartitionSpec("core")
    in_specs = (spec,) * (n_params + n_outs)
    out_specs = (spec,) * n_outs
    donate = tuple(range(n_params, n_params + n_outs))
    sharded = jax.jit(
        shard_map(_body, mesh=mesh, in_specs=in_specs, out_specs=out_specs,
                  check_rep=False),
        donate_argnums=donate, keep_unused=True,
    )
    concat_in = [
        np.concatenate([np.asarray(in_maps[c][nm]) for c in range(n_cores)], axis=0)
        for nm in in_names
    ]
    shd = NamedSharding(mesh, spec)
    dev_in = [jax.device_put(x, shd) for x in concat_in]

    def fresh_zeros():
        return [jax.device_put(
            np.zeros((n_cores * z.shape[0], *z.shape[1:]), z.dtype), shd)
            for z in zero_outs]

    times = []
    for it in range(iters):
        zs = fresh_zeros()
        for z in zs:
            z.block_until_ready()
        t0 = time.perf_counter()
        outs = sharded(*dev_in, *zs)
        for o in outs:
            o.block_until_ready()
        t1 = time.perf_counter()
        times.append(t1 - t0)
        del outs
    return min(times), times


def kernel(C, Q, c_mask, q_mask, w):
    C = np.asarray(C, dtype=np.float32)
    Q = np.asarray(Q, dtype=np.float32)
    res = run_on_cores(C, Q, w)
    out = np.concatenate([res.results[c]["out"] for c in range(N_CORES)], axis=0)
    return out


if __name__ == "__main__":
    np.random.seed(0)
    nb = int(sys.argv[1]) if len(sys.argv) > 1 else 1
    ncore = int(sys.argv[2]) if len(sys.argv) > 2 else 1
    B = nb * ncore
    C = np.random.randn(B, D, LC).astype(np.float32)
    Q = np.random.randn(B, D, LQ).astype(np.float32)
    lim = np.sqrt(1.0 / D)
    w = np.random.uniform(-lim, lim, 3 * D).astype(np.float32)

    res = run_on_cores(C, Q, w, n_batches=nb, n_cores=ncore)
    got = np.concatenate([res.results[c]["out"] for c in range(ncore)], axis=0)

    # numpy reference
    outs = []
    for b in range(B):
        Ct = C[b].T.astype(np.float64)
        Qt = Q[b].T.astype(np.float64)
        w1, w2, w3 = w[:D].astype(np.float64), w[D:2*D].astype(np.float64), w[2*D:].astype(np.float64)
        S = (Ct * w3) @ Qt.T + (Ct @ w1)[:, None] + (Qt @ w2)[None, :]
        E = np.exp(S - S.max(1, keepdims=True))
        S1 = E / E.sum(1, keepdims=True)
        E2 = np.exp(S - S.max(0, keepdims=True))
        S2 = E2 / E2.sum(0, keepdims=True)
        A = S1 @ Qt
        Bm = (S1 @ S2.T) @ Ct
        outs.append(np.concatenate([Ct, A, Ct * A, Ct * Bm], axis=1).T)
    ref = np.stack(outs)
    d = np.abs(got - ref)
    denom = np.abs(ref) + 1e-6
    print(f"max_abs={d.max():.3e} max_rel={(d/denom).max():.3e} "
          f"norm_rel={np.linalg.norm(got-ref)/np.linalg.norm(ref):.3e}")
    for qi in range(4):
        g = got[:, qi*256:(qi+1)*256]; e = ref[:, qi*256:(qi+1)*256]
        print(f"  quarter {qi}: max_abs={np.abs(g-e).max():.3e} "
              f"norm_rel={np.linalg.norm(g-e)/max(np.linalg.norm(e),1e-9):.3e}")



# revision 5
# speedup vs baseline: 1.8249x; 1.8249x over previous
"""CQAttention Trainium2 kernel (v2: bf16, single-exp flow).

Full inputs: C (64,256,1024), Q (64,256,256), c_mask (64,1024) [all-ones],
q_mask (64,256) [all-ones], w (768,).  Output: (64, 1024, 1024) fp32.

Sharding: data-parallel over batch, 8 batches per core on 8 cores.

Math per batch (Ct = C^T (c,d), Qt = Q^T (q,d)):
  S[c,q] = b1[c] + b2[q] + s_core[c,q],  s_core = (Ct*w3) @ Qt^T
  S1 = softmax_q(S) = G / r,   G = exp(s_core + b2), r[c] = sum_q G
       (the per-row factor e^{b1[c]} cancels)
  S2 = softmax_c(S);  T = S2^T @ Ct = U[:, :d] / U[:, d]
       with U = G^T @ Ctb, Ctb = e^{b1[c]} * [Ct | 1]
       (the per-col factor e^{b2[q]} cancels inside T)
  A  = S1 @ Qt      -> A^T  = Qt^T @ S1^T   (S1^T via PE transpose)
  Bm = S1 @ T       -> Bm^T = T^T @ S1^T
  out = [Ct; A; Ct*A; Ct*Bm]^T; the first d rows equal C and are
  filled on the host; the device emits o2=A^T, o3=C.A^T, o4=C.Bm^T in bf16.

All matmuls in bf16 (full PE rate; fp32r runs at half rate).
"""

import sys

for _p in ("/opt/trn_rl_repo",):
    if _p not in sys.path:
        sys.path.insert(0, _p)

import numpy as np
import ml_dtypes
from contextlib import ExitStack

import concourse.bass as bass
import concourse.mybir as mybir
import concourse.tile as tile
from concourse import masks
from concourse.bass_utils import run_bass_kernel_spmd

F32 = mybir.dt.float32
BF16 = mybir.dt.bfloat16
EXP = mybir.ActivationFunctionType.Exp
BF = ml_dtypes.bfloat16

N_CORES = 8
B_FULL, D, LC, LQ = 64, 256, 1024, 256
BPC = B_FULL // N_CORES  # batches per core
KT = D // 128            # 2 d-tiles
CT_N = LC // 128         # 8 c-tiles
QT_N = LQ // 128         # 2 q-tiles
CW = D + 2               # Ctb row width (Ct | e^{b1} | pad)

LA_W = KT * LC + KT * LQ          # 2048 + 512 = 2560   (C d-major, Q d-major)
LB_W = CT_N * CW + QT_N * D       # 2064 + 512 = 2576   (Ctb c-major, Qt q-major)


def split_multi_waits(nc):
    """Walrus in this container accepts at most one sync-wait command per
    instruction; hoist extras onto single-wait drain nops just before."""
    n_new = 0
    for fn in nc.m.functions:
        for blk in fn.blocks:
            out_list = []
            changed = False
            for inst in blk.instructions:
                si = inst.sync_info
                if si is not None and si.on_wait and len(si.on_wait) > 1:
                    waits = list(si.on_wait)
                    for w in waits[:-1]:
                        nop = mybir.InstDrain(
                            name=f"I-waitsplit-{n_new}", ins=[], outs=[]
                        )
                        n_new += 1
                        nop.engine = inst.engine
                        nop.sync_info = mybir.SyncInfo(on_wait=[w], on_update=[])
                        out_list.append(nop)
                    inst.sync_info = mybir.SyncInfo(
                        on_wait=[waits[-1]], on_update=list(si.on_update)
                    )
                    changed = True
                out_list.append(inst)
            if changed:
                blk.instructions = out_list
    return n_new


def build_module(n_batches=BPC, rounds=1):
    nc = bass.Bass()
    la_d = nc.declare_dram_parameter("la", [n_batches, 128, LA_W], BF16, isOutput=False)
    lb_d = nc.declare_dram_parameter("lb", [n_batches, 128, LB_W], BF16, isOutput=False)
    b2r_d = nc.declare_dram_parameter("b2r", [n_batches, 1, LQ], BF16, isOutput=False)
    w3c_d = nc.declare_dram_parameter("w3c", [128, KT], F32, isOutput=False)
    out_d = nc.declare_dram_parameter(
        "outp", [n_batches, 3, KT, 128, LC], BF16, isOutput=True
    )

    with tile.TileContext(nc) as tc, ExitStack() as ctx:
        cpool = ctx.enter_context(tc.tile_pool(name="const", bufs=1))
        spool = ctx.enter_context(tc.tile_pool(name="sbuf", bufs=2))
        ppool = ctx.enter_context(tc.tile_pool(name="psum", bufs=2, space="PSUM"))

        # ---- per-core constants ----
        w3c = cpool.tile([128, KT], F32, name="w3c")
        nc.sync.dma_start(w3c[:], w3c_d[:])
        onesA = cpool.tile([1, 128], BF16, name="onesA")
        nc.vector.memset(onesA[:], 1.0)
        ident = cpool.tile([128, 128], BF16, name="ident")
        masks.make_identity(nc, ident[:])

        for _round in range(rounds):
          for b in range(n_batches):
            # ---------------- loads ----------------
            la = spool.tile([128, LA_W], BF16, name="la", tag="la", bufs=3)
            nc.sync.dma_start(la[:], la_d[b])
            lb = spool.tile([128, LB_W], BF16, name="lb", tag="lb", bufs=3)
            nc.sync.dma_start(lb[:], lb_d[b])
            b2row = spool.tile([1, LQ], BF16, name="b2row", tag="b2row", bufs=3)
            nc.sync.dma_start(b2row[:], b2r_d[b])

            def Ck(k, lo=0, hi=LC):      # C d-major, k-th 128-row slab
                return la[:, k * LC + lo : k * LC + hi]

            def Qf(k):                   # Q d-major
                return la[:, KT * LC + k * LQ : KT * LC + (k + 1) * LQ]

            def Ctb(i):                  # Ctb c-major tile i (128 x 258)
                return lb[:, i * CW : (i + 1) * CW]

            def Qt(qt, dlo, dhi):        # Qt q-major
                base = CT_N * CW + qt * D
                return lb[:, base + dlo : base + dhi]

            # ---------------- Cw3 = C * w3 (bf16) ----------------
            cw3 = spool.tile([128, KT * LC], BF16, name="cw3", tag="cw3")
            for k in range(KT):
                nc.vector.tensor_scalar_mul(
                    cw3[:, k * LC : (k + 1) * LC], Ck(k), w3c[:, k : k + 1]
                )

            # ---------------- G = exp(s_core + b2), r = rowsum ----------------
            G = spool.tile([128, CT_N, LQ], BF16, name="G", tag="G")
            r = spool.tile([128, CT_N], F32, name="r", tag="r")
            invr = spool.tile([128, CT_N], F32, name="invr", tag="invr")
            for i in range(CT_N):
                ps = ppool.tile([128, LQ], F32, name="ps", tag="g", bufs=2)
                for k in range(KT):
                    nc.tensor.matmul(
                        ps[:],
                        cw3[:, k * LC + i * 128 : k * LC + (i + 1) * 128],
                        Qf(k),
                        start=(k == 0),
                        stop=False,
                    )
                nc.tensor.matmul(ps[:], onesA[:], b2row[:], start=False, stop=True)
                nc.scalar.activation(
                    G[:, i, :], ps[:], EXP, accum_out=r[:, i : i + 1]
                )

            # ---------------- S1 = G / r ----------------
            nc.vector.reciprocal(invr[:], r[:])
            S1 = spool.tile([128, CT_N, LQ], BF16, name="S1", tag="S1")
            for i in range(CT_N):
                nc.vector.tensor_scalar_mul(
                    S1[:, i, :], G[:, i, :], invr[:, i : i + 1]
                )

            # ---------------- S1t = S1^T via PE transpose ----------------
            S1t = spool.tile([128, QT_N, LC], BF16, name="S1t", tag="S1t")
            for qt in range(QT_N):
                pt = ppool.tile([128, LC], BF16, name="pt", tag="t", bufs=1)
                for i in range(CT_N):
                    nc.tensor.transpose(
                        pt[:, i * 128 : (i + 1) * 128],
                        S1[:, i, qt * 128 : (qt + 1) * 128],
                        ident[:],
                    )
                nc.scalar.copy(S1t[:, qt, :], pt[:])

            # ---------------- U = G^T @ Ctb -> T = U/s ----------------
            T = spool.tile([128, QT_N, D], BF16, name="T", tag="T")
            invs = spool.tile([128, QT_N], F32, name="invs", tag="invs")
            for qt in range(QT_N):
                pu = ppool.tile([128, CW], F32, name="pu", tag="u", bufs=1)
                for i in range(CT_N):
                    nc.tensor.matmul(
                        pu[:],
                        G[:, i, qt * 128 : (qt + 1) * 128],
                        Ctb(i),
                        start=(i == 0),
                        stop=(i == CT_N - 1),
                    )
                nc.vector.reciprocal(invs[:, qt : qt + 1], pu[:, D : D + 1])
                nc.vector.tensor_scalar_mul(
                    T[:, qt, :], pu[:, 0:D], invs[:, qt : qt + 1]
                )

            # ---------------- outputs: o2=A^T, o3=C*A^T, o4=C*Bm^T ----------------
            ost = spool.tile([128, 3, KT, LC], BF16, name="ost", tag="ost")
            for dt in range(KT):
                pa = [
                    ppool.tile([128, 512], F32, name=f"pa{nh}", tag="ab", bufs=4)
                    for nh in range(2)
                ]
                for qt in range(QT_N):
                    for nh in range(2):
                        nc.tensor.matmul(
                            pa[nh][:],
                            Qt(qt, dt * 128, (dt + 1) * 128),
                            S1t[:, qt, nh * 512 : (nh + 1) * 512],
                            start=(qt == 0),
                            stop=(qt == QT_N - 1),
                        )
                for nh in range(2):
                    nc.scalar.copy(
                        ost[:, 0, dt, nh * 512 : (nh + 1) * 512], pa[nh][:]
                    )
                    nc.vector.tensor_mul(
                        ost[:, 1, dt, nh * 512 : (nh + 1) * 512],
                        Ck(dt, nh * 512, (nh + 1) * 512),
                        ost[:, 0, dt, nh * 512 : (nh + 1) * 512],
                    )

            for dt in range(KT):
                pm = [
                    ppool.tile([128, 512], F32, name=f"pm{nh}", tag="ab", bufs=4)
                    for nh in range(2)
                ]
                for qt in range(QT_N):
                    for nh in range(2):
                        nc.tensor.matmul(
                            pm[nh][:],
                            T[:, qt, dt * 128 : (dt + 1) * 128],
                            S1t[:, qt, nh * 512 : (nh + 1) * 512],
                            start=(qt == 0),
                            stop=(qt == QT_N - 1),
                        )
                for nh in range(2):
                    nc.vector.tensor_mul(
                        ost[:, 2, dt, nh * 512 : (nh + 1) * 512],
                        Ck(dt, nh * 512, (nh + 1) * 512),
                        pm[nh][:],
                    )

            nc.gpsimd.dma_start(
                out_d[b].rearrange("w k p c -> p w k c"), ost[:]
            )

    split_multi_waits(nc)
    return nc


def host_prep(C, Q, w):
    """Host-side packing: transposes, bias folds, bf16 rounding."""
    B = C.shape[0]
    w1, w2, w3 = w[:D], w[D:2 * D], w[2 * D:]
    b1 = np.einsum("bdc,d->bc", C, w1).astype(np.float32)   # (B, LC)
    b2 = np.einsum("bdq,d->bq", Q, w2).astype(np.float32)   # (B, LQ)
    eb1 = np.exp(b1)                                        # (B, LC)

    # la: [C d-major (128, 2*1024) | Q d-major (128, 2*256)]
    la = np.empty((B, 128, LA_W), dtype=BF)
    la[:, :, : KT * LC] = (
        C.reshape(B, KT, 128, LC).transpose(0, 2, 1, 3).reshape(B, 128, KT * LC)
    ).astype(BF)
    la[:, :, KT * LC :] = (
        Q.reshape(B, KT, 128, LQ).transpose(0, 2, 1, 3).reshape(B, 128, KT * LQ)
    ).astype(BF)

    # lb: [Ctb c-major (128, 8*258) | Qt q-major (128, 2*256)]
    Ct = C.transpose(0, 2, 1)                               # (B, LC, D)
    Ctb = np.empty((B, LC, CW), dtype=np.float32)
    Ctb[:, :, :D] = Ct * eb1[:, :, None]
    Ctb[:, :, D] = eb1
    Ctb[:, :, D + 1] = 0.0
    lb = np.empty((B, 128, LB_W), dtype=BF)
    lb[:, :, : CT_N * CW] = (
        Ctb.reshape(B, CT_N, 128, CW).transpose(0, 2, 1, 3).reshape(B, 128, CT_N * CW)
    ).astype(BF)
    Qt = Q.transpose(0, 2, 1)                               # (B, LQ, D)
    lb[:, :, CT_N * CW :] = (
        Qt.reshape(B, QT_N, 128, D).transpose(0, 2, 1, 3).reshape(B, 128, QT_N * D)
    ).astype(BF)

    b2r = b2[:, None, :].astype(BF)                         # (B, 1, LQ)

    w3c = np.zeros((128, KT), dtype=np.float32)
    for k in range(KT):
        w3c[:, k] = w3[k * 128 : (k + 1) * 128]
    return dict(la=la, lb=lb, b2r=b2r), w3c


_NC_CACHE = {}


def _get_module(n_batches=BPC, rounds=1):
    key = (n_batches, rounds)
    if key not in _NC_CACHE:
        _NC_CACHE[key] = build_module(n_batches, rounds)
    return _NC_CACHE[key]


def run_on_cores(C, Q, w, n_batches=BPC, n_cores=N_CORES, **spmd_kwargs):
    nc = _get_module(n_batches)
    prep, w3c = host_prep(np.asarray(C, np.float32), np.asarray(Q, np.float32),
                          np.asarray(w, np.float32))
    in_maps = []
    for c in range(n_cores):
        sl = slice(c * n_batches, (c + 1) * n_batches)
        m = {"w3c": w3c}
        for k in ("la", "lb", "b2r"):
            m[k] = np.ascontiguousarray(prep[k][sl])
        in_maps.append(m)
    res = run_bass_kernel_spmd(nc, in_maps, list(range(n_cores)), **spmd_kwargs)
    return res


def assemble(C, res, n_batches=BPC, n_cores=N_CORES):
    B = n_cores * n_batches
    out = np.empty((B, 4 * D, LC), dtype=np.float32)
    out[:, :D, :] = C
    for c in range(n_cores):
        dev = np.asarray(res.results[c]["outp"])          # (nb, 3, KT, 128, LC) bf16
        sl = slice(c * n_batches, (c + 1) * n_batches)
        out[sl, D:, :] = dev.reshape(n_batches, 3 * D, LC).astype(np.float32)
    return out


def kernel(C, Q, c_mask, q_mask, w):
    C = np.asarray(C, dtype=np.float32)
    Q = np.asarray(Q, dtype=np.float32)
    res = run_on_cores(C, Q, w)
    return assemble(C, res)


if __name__ == "__main__":
    np.random.seed(0)
    nb = int(sys.argv[1]) if len(sys.argv) > 1 else 1
    ncore = int(sys.argv[2]) if len(sys.argv) > 2 else 1
    B = nb * ncore
    C = np.random.randn(B, D, LC).astype(np.float32)
    Q = np.random.randn(B, D, LQ).astype(np.float32)
    lim = np.sqrt(1.0 / D)
    w = np.random.uniform(-lim, lim, 3 * D).astype(np.float32)

    res = run_on_cores(C, Q, w, n_batches=nb, n_cores=ncore)
    got = assemble(C, res, n_batches=nb, n_cores=ncore)

    # numpy reference
    outs = []
    for b in range(B):
        Ct = C[b].T.astype(np.float64)
        Qt = Q[b].T.astype(np.float64)
        w1, w2, w3 = w[:D].astype(np.float64), w[D:2*D].astype(np.float64), w[2*D:].astype(np.float64)
        S = (Ct * w3) @ Qt.T + (Ct @ w1)[:, None] + (Qt @ w2)[None, :]
        E = np.exp(S - S.max(1, keepdims=True))
        S1 = E / E.sum(1, keepdims=True)
        E2 = np.exp(S - S.max(0, keepdims=True))
        S2 = E2 / E2.sum(0, keepdims=True)
        A = S1 @ Qt
        Bm = (S1 @ S2.T) @ Ct
        outs.append(np.concatenate([Ct, A, Ct * A, Ct * Bm], axis=1).T)
    ref = np.stack(outs)
    d = np.abs(got - ref)
    denom = np.abs(ref) + 1e-6
    print(f"max_abs={d.max():.3e} max_rel={(d/denom).max():.3e} "
          f"norm_rel={np.linalg.norm(got-ref)/np.linalg.norm(ref):.3e}")
    for qi in range(4):
        g = got[:, qi*256:(qi+1)*256]; e = ref[:, qi*256:(qi+1)*256]
        print(f"  quarter {qi}: max_abs={np.abs(g-e).max():.3e} "
              f"norm_rel={np.linalg.norm(g-e)/max(np.linalg.norm(e),1e-9):.3e}")
